# revision 3
# baseline (speedup 1.0000x reference)
"""Trainium2 Bass kernel: ContextCrossAttention (B,C,H,W)=(8,512,128,128).

Math per batch element b (algebraically collapsed from the reference):
  q      = Wq @ ctx_b + bq                          (C,)
  qks    = (q @ Wk) * C**-0.5                       (C,)
  p[hw]  = exp(logits[hw]);  Z = sum(p)             (softmax shift dropped)
  pooled = x_b @ p                                  (C,)
  gate   = (Wv @ pooled) / Z + bv                   (C,)
  out_b  = x_b * gate[:, None]

Sharding: pure data-parallel over batch; core i handles batch element i.
x streams in as bf16 (16 MiB/core), stays resident in SBUF, and the
output is written bf16 (upcast on host) -- the kernel is HBM-bound at
~410 GB/s/core, so phase 1 (stream + logits/exp/pooled) and phase 3
(out = x*gate) are each ~42 us of DMA with a short compute drain between.

The pooled-pass product+reduce work is spread over three engines so the
per-group cadence tracks the 5.25 us/group DMA stream (measured rates:
DVE STT-with-accum 2.3 us / TT-mult 1.2 us per [128,2048] bf16 tile;
ACT copy-accum 2.3 us incl. accumulator read; exps 22 us total fixed):
  - cc0 "stt":    DVE scalar_tensor_tensor w/ accum (1x mode); on two
    early groups it shifts to tt_act to rebalance DVE vs ACT
  - cc1 "tt_act": DVE tensor_mul (2x mode) + ACT copy-accum
  - cc2/cc3 "tt_pe": DVE tensor_mul + PE identity-stationary matmuls
    accumulating into a [128,1024] PSUM accumulator; consecutive
    accumulates alternate bank halves so the PE pipeline never stalls on
    the accumulate RAW hazard; one ACT copy-accum reads it at finalize.

Structural findings baked in:
  - GpSimd is useless here: tensor ops run at 0.28-0.42x roofline and a
    concurrent Pool op degrades every DVE op 1.84x (shared SBUF ports).
  - The DMA ring serializes its tail onto one engine (~25 GB/s runt), so
    the sync ring is issued in PROCESSING order with the runt victim
    processed last; weights ride the scalar queue (never the sync ring,
    and never ahead of ACT work, which shares the scalar queue).
  - All reductions that can close early do: Z and the first 7 groups of
    pcols pre-reduce during streaming; gate uses one [128,4] PSUM tile
    and a single fused STT; the last two output tiles split their DMA in
    half so the output ring drains in parallel.
"""

import numpy as np
import ml_dtypes
from contextlib import ExitStack

import concourse.bass as bass
import concourse.bacc as bacc
import concourse.tile as tile
from concourse import mybir
from concourse.bass_utils import run_bass_kernel_spmd

F32 = mybir.dt.float32
BF16 = mybir.dt.bfloat16
AF = mybir.ActivationFunctionType
OP = mybir.AluOpType

B, C, D, H, W = 8, 512, 512, 128, 128
HW = H * W                      # 16384
P = 128                         # partitions
CCH = C // P                    # 4 channel chunks
NCORES = 8
G = 8                           # hw groups
GW = HW // G                    # 2048 group width
SCALE = float(C) ** -0.5

PGW = 1024                      # psum logits group width (2 banks each)
NH = GW // PGW                  # 2 psum halves per group

ROUTES = ("stt", "tt_act", "tt_pe", "tt_pe")
WQKB_W = CCH * C + 2 * CCH + P  # wqk chunks | ctx | bqk | identity


def _build_kernel():
    nc = bacc.Bacc(
        "TRN2",
        target_bir_lowering=False,
        debug=False,
        enable_asserts=False,
        num_devices=NCORES,
    )

    xd = nc.dram_tensor("xb", [C, HW], BF16, kind="ExternalInput")
    wqkbd = nc.dram_tensor("wqkb", [P, WQKB_W], BF16, kind="ExternalInput")
    wvtd = nc.dram_tensor("wvt", [P, CCH * C], BF16, kind="ExternalInput")
    bvd = nc.dram_tensor("bvc", [P, CCH], F32, kind="ExternalInput")
    outd = nc.dram_tensor("out", [C, HW], BF16, kind="ExternalOutput")

    pe_ccs = [cc for cc, r in enumerate(ROUTES) if r == "tt_pe"]

    with tile.TileContext(nc) as tc, ExitStack() as ctx:
        singles = ctx.enter_context(tc.tile_pool(name="singles", bufs=1))
        xt = ctx.enter_context(tc.tile_pool(name="xt", bufs=G * CCH))
        prods = ctx.enter_context(tc.tile_pool(name="prods", bufs=2))
        outp = ctx.enter_context(tc.tile_pool(name="outp", bufs=5))
        psb = ctx.enter_context(tc.tile_pool(name="psb", bufs=3))
        finp = ctx.enter_context(tc.tile_pool(name="finp", bufs=1))

        # ---- weights prefix on the scalar queue; x stream owns sync ----
        wqkb_sb = singles.tile([P, WQKB_W], BF16, tag="wqkb", name="wqkb")
        nc.scalar.dma_start(wqkb_sb[:], wqkbd[:])
        wvt_sb = singles.tile([P, CCH * C], BF16, tag="wvt", name="wvt")
        nc.scalar.dma_start(wvt_sb[:], wvtd[:])
        bv_sb = singles.tile([P, CCH], F32, tag="bv", name="bv")
        nc.scalar.dma_start(bv_sb[:], bvd[:])
        ident = wqkb_sb[:, CCH * C + 2 * CCH:CCH * C + 2 * CCH + P]

        ones128 = singles.tile([P, P], BF16, tag="ones128")
        nc.vector.memset(ones128[:], 1.0)

        qks_sb = singles.tile([P, CCH], F32, tag="qks")
        qksb = [singles.tile([P, P], BF16, tag=f"qksb{cc}", name=f"qksb{cc}") for cc in range(CCH)]
        pooled_sb = singles.tile([P, CCH], BF16, tag="pooled")
        pooled_f32 = singles.tile([P, CCH], F32, tag="pooledf")
        gate_sb = singles.tile([P, CCH], F32, tag="gate")
        zcols = singles.tile([P, G * NH], F32, tag="zcols")
        pcols = [singles.tile([P, G], F32, tag=f"pcols{cc}", name=f"pcols{cc}") for cc in range(CCH)]
        z_sb = singles.tile([P, 1], F32, tag="z")
        z2_sb = singles.tile([P, 1], F32, tag="z2")
        rz_sb = singles.tile([P, 1], F32, tag="rz")

        # ---- x stream: bulk on the sync queue; the tail groups split into
        # half-tiles across both queues so the ring-drain runt is small ----
        # one sync ring, issued in processing order: the ring's slow
        # tail-drain (runt) then only delays the group processed last.
        GORDER = [0, 1, 2, 3, 4, 7, 6, 5]
        x_tiles = {}
        for g in GORDER:
            for cc in range(CCH):
                t = xt.tile([P, GW], BF16, tag="x", name="x_t")
                nc.sync.dma_start(t[:], xd[cc * P:(cc + 1) * P, g * GW:(g + 1) * GW])
                x_tiles[(cc, g)] = t

        # ---- qks = ctx @ Wqk + bqk (pre-scaled); qksB replicated 128x ----
        with tc.tile_pool(name="pssm1", bufs=2, space="PSUM") as pssm1:
            for cc in range(CCH):
                pqk = pssm1.tile([P, 1], F32, tag="pssm", name="pssm_t")
                for dc in range(CCH):
                    nc.tensor.matmul(
                        pqk[:], wqkb_sb[:, dc * C + cc * P:dc * C + (cc + 1) * P],
                        wqkb_sb[:, CCH * C + dc:CCH * C + dc + 1],
                        start=(dc == 0), stop=(dc == CCH - 1),
                    )
                nc.vector.tensor_add(
                    qks_sb[:, cc:cc + 1], pqk[:],
                    wqkb_sb[:, CCH * C + CCH + cc:CCH * C + CCH + cc + 1])
                nc.vector.tensor_scalar_mul(qksb[cc][:], ones128[:], qks_sb[:, cc:cc + 1])

        # ---- fused pass: logits -> exp -> products, one group pipelined ----
        with tc.tile_pool(name="pslog", bufs=2, space="PSUM") as pslog, \
             tc.tile_pool(name="psacc", bufs=1, space="PSUM") as psacc:
            accs = {cc: psacc.tile([P, 2 * 512], F32, tag=f"acc{cc}", name=f"acc{cc}")
                    for cc in pe_ccs}
            half_mm_count = {(cc, hb): 0 for cc in pe_ccs for hb in (0, 1)}
            HALF_MM_TOTAL = G * (GW // 512) // 2

            pr_tiles = {}
            pe_tiles = {}

            def _act_reduce(pos):
                for cc in range(CCH):
                    if (cc, pos) in pr_tiles:
                        pr = pr_tiles.pop((cc, pos))
                        nc.scalar.activation(
                            pr[:], pr[:], AF.Copy,
                            accum_out=pcols[cc][:, pos:pos + 1],
                        )

            def _pe_acc(pos):
                for s in range(GW // 512):
                    hb = s % 2
                    for cc in pe_ccs:
                        pr = pe_tiles[(cc, pos)]
                        i = half_mm_count[(cc, hb)]
                        nc.tensor.matmul(
                            accs[cc][:, hb * 512:(hb + 1) * 512],
                            ident, pr[:, s * 512:(s + 1) * 512],
                            start=(i == 0), stop=(i == HALF_MM_TOTAL - 1),
                            skip_group_check=True,
                        )
                        half_mm_count[(cc, hb)] += 1
                for cc in pe_ccs:
                    pe_tiles.pop((cc, pos))

            prev_g = None
            for pos, g in enumerate(GORDER):
                p_t = psb.tile([P, GW], BF16, tag="p", name="p_t")
                for h in range(NH):
                    gh = pos * NH + h
                    plog = pslog.tile([P, PGW], F32, tag="plog", name="plog_t")
                    for s in range(PGW // 512):
                        for cc in range(CCH):
                            nc.tensor.matmul(
                                plog[:, s * 512:(s + 1) * 512],
                                qksb[cc][:],
                                x_tiles[(cc, g)][:, h * PGW + s * 512:h * PGW + (s + 1) * 512],
                                start=(cc == 0), stop=(cc == CCH - 1),
                            )
                    nc.scalar.activation(
                        p_t[:, h * PGW:(h + 1) * PGW], plog[:], AF.Exp,
                        accum_out=zcols[:, gh:gh + 1],
                    )
                routes = list(ROUTES)
                if pos in (1, 3):
                    routes[0] = "tt_act"      # rebalance DVE->ACT early on
                if pos == G - 1:
                    routes[1] = "stt"         # no ACT copy in the drain
                for cc in range(CCH):
                    if routes[cc] != "stt":
                        pr = prods.tile([P, GW], BF16, tag=f"pr{cc}", name=f"pr{cc}_t")
                        nc.vector.tensor_mul(pr[:], x_tiles[(cc, g)][:], p_t[:])
                        if routes[cc] == "tt_act":
                            pr_tiles[(cc, pos)] = pr
                        else:
                            pe_tiles[(cc, pos)] = pr
                stt_ccs = [cc for cc in range(CCH) if routes[cc] == "stt"]
                for k, cc in enumerate(stt_ccs):
                    # the LAST stt's (discarded) output overwrites the dead
                    # p_t; earlier ones write a scratch prods tile
                    if k == len(stt_ccs) - 1:
                        gout = p_t
                    else:
                        gout = prods.tile([P, GW], BF16, tag="pr1", name="pr1_t")
                    nc.vector.scalar_tensor_tensor(
                        gout[:], x_tiles[(cc, g)][:], 1.0, p_t[:],
                        op0=OP.mult, op1=OP.mult,
                        accum_out=pcols[cc][:, pos:pos + 1],
                    )
                if prev_g is not None:
                    _act_reduce(prev_g)
                    _pe_acc(prev_g)
                if pos == G - 2:
                    # pre-reduce the first seven positions while the last
                    # group is still streaming/processing
                    nc.vector.reduce_sum(
                        z_sb[:], zcols[:, 0:(G - 1) * NH], axis=mybir.AxisListType.X)
                prev_g = pos
            _act_reduce(prev_g)
            _pe_acc(prev_g)

            # ---- finalize: Z, pooled ----
            nc.vector.reduce_sum(
                z2_sb[:], zcols[:, (G - 1) * NH:], axis=mybir.AxisListType.X)
            nc.vector.tensor_add(z_sb[:], z_sb[:], z2_sb[:])
            nc.vector.reciprocal(rz_sb[:], z_sb[:])
            for cc in range(CCH):
                if ROUTES[cc] == "tt_pe":
                    sa = finp.tile([P, 2 * 512], F32, tag="fin", name="fin_t")
                    nc.scalar.activation(
                        sa[:], accs[cc][:], AF.Copy,
                        accum_out=pooled_f32[:, cc:cc + 1],
                    )
                else:
                    nc.vector.reduce_sum(
                        pooled_f32[:, cc:cc + 1], pcols[cc][:],
                        axis=mybir.AxisListType.X,
                    )
            nc.vector.tensor_copy(pooled_sb[:], pooled_f32[:])

        # ---- gate = (Wv @ pooled)/Z + bv ----
        with tc.tile_pool(name="pssm2", bufs=1, space="PSUM") as pssm2:
            pg = pssm2.tile([P, CCH], F32, tag="pssm", name="pssm_t")
            for oc in range(CCH):
                for cc in range(CCH):
                    nc.tensor.matmul(
                        pg[:, oc:oc + 1],
                        wvt_sb[:, cc * C + oc * P:cc * C + (oc + 1) * P],
                        pooled_sb[:, cc:cc + 1],
                        start=(cc == 0), stop=(cc == CCH - 1),
                        skip_group_check=True,
                    )
            nc.vector.scalar_tensor_tensor(
                gate_sb[:], pg[:], rz_sb[:], bv_sb[:],
                op0=OP.mult, op1=OP.add,
            )

        # ---- pass C: out = x * gate (all of x still resident in SBUF) ----
        engs = [nc.sync, nc.scalar]
        for idx in range(G * CCH):
            g, cc = divmod(idx, CCH)
            o = outp.tile([P, GW], BF16, tag="o", name="o_t")
            nc.vector.tensor_scalar_mul(o[:], x_tiles[(cc, g)][:], gate_sb[:, cc:cc + 1])
            if idx < G * CCH - 2:
                engs[idx % 2].dma_start(outd[cc * P:(cc + 1) * P, g * GW:(g + 1) * GW], o[:])
            else:
                # halve the final transfers so the ring-drain runt is short
                for hh in range(2):
                    engs[(idx + hh) % 2].dma_start(
                        outd[cc * P:(cc + 1) * P,
                             g * GW + hh * (GW // 2):g * GW + (hh + 1) * (GW // 2)],
                        o[:, hh * (GW // 2):(hh + 1) * (GW // 2)])

    nc.compile()
    return nc


_NC = None


def _get_nc():
    global _NC
    if _NC is None:
        _NC = _build_kernel()
    return _NC


def _chunk_major(w):
    # w_c[p, j*C + k] = w[j*128 + p, k]
    w = np.asarray(w, dtype=np.float32).reshape(CCH, P, C)
    return np.ascontiguousarray(w.transpose(1, 0, 2).reshape(P, CCH * C))


def _make_in_maps(x, context, Wq, bq, Wk, bk, Wv, bv):
    bf = ml_dtypes.bfloat16
    x = np.asarray(x, dtype=np.float32).reshape(B, C, HW).astype(bf)
    Wq = np.asarray(Wq, dtype=np.float32)
    Wk = np.asarray(Wk, dtype=np.float32)
    wqk = _chunk_major(Wq.T @ Wk * SCALE).astype(bf)
    bqk = (np.asarray(bq, dtype=np.float32) @ Wk) * SCALE
    wvt = _chunk_major(np.asarray(Wv, dtype=np.float32).T).astype(bf)
    bqkc = np.ascontiguousarray(bqk.reshape(CCH, P).T).astype(bf)
    bvc = np.ascontiguousarray(np.asarray(bv, dtype=np.float32).reshape(CCH, P).T)
    identc = np.eye(P, dtype=np.float32).astype(bf)
    context = np.asarray(context, dtype=np.float32)
    in_maps = []
    for b in range(NCORES):
        ctxc = np.ascontiguousarray(context[b].reshape(CCH, P).T).astype(bf)
        wqkb = np.ascontiguousarray(
            np.concatenate([wqk, ctxc, bqkc, identc], axis=1))
        in_maps.append({
            "xb": x[b],
            "wqkb": wqkb,
            "wvt": wvt,
            "bvc": bvc,
        })
    return in_maps


def run_spmd(x, context, Wq, bq, Wk, bk, Wv, bv, **spmd_kwargs):
    """Run on 8 NeuronCores; returns (output (B,C,H,W) f32, BassKernelResults)."""
    nc = _get_nc()
    in_maps = _make_in_maps(x, context, Wq, bq, Wk, bk, Wv, bv)
    res = run_bass_kernel_spmd(nc, in_maps, list(range(NCORES)), **spmd_kwargs)
    out = np.stack([
        np.asarray(res.results[b]["out"]).astype(np.float32).reshape(C, H, W)
        for b in range(NCORES)
    ])
    return out, res


def kernel(x, context, Wq, bq, Wk, bk, Wv, bv):
    out, _ = run_spmd(x, context, Wq, bq, Wk, bk, Wv, bv)
    return out


# revision 4
# speedup vs baseline: 1.0787x; 1.0787x over previous
"""Trainium2 Bass kernel: ContextCrossAttention (B,C,H,W)=(8,512,128,128).

Math per batch element b (algebraically collapsed from the reference):
  q      = Wq @ ctx_b + bq                          (C,)
  qks    = (q @ Wk) * C**-0.5                       (C,)
  p[hw]  = exp(logits[hw]);  Z = sum(p)             (softmax shift dropped)
  pooled = x_b @ p                                  (C,)
  gate   = (Wv @ pooled) / Z + bv                   (C,)
  out_b  = x_b * gate[:, None]

Sharding: pure data-parallel over batch; core i handles batch element i.

The pooled-pass product+reduce work is spread over three engines so the
per-group cadence tracks the x DMA stream (5.25us/group):
  - cc0 "stt":    DVE scalar_tensor_tensor w/ accum (2.3us, 1x mode)
  - cc1 "tt_act": DVE tensor_mul (1.2us, 2x mode) + ACT copy-accum (2.0us)
  - cc2/cc3 "tt_pe": DVE tensor_mul + PE identity-stationary matmuls that
    accumulate the product into a [128,1024] PSUM accumulator (2 banks,
    alternating halves so consecutive accumulates hit different banks and
    keep the PE pipelined); one ACT copy-accum at finalize reads it.

PSUM is fully packed (plog 4 banks + 2x acc 2 banks); the tiny [P,1]
matmul tiles for qks/gate live in pools scoped before/after phase 1.
"""

import numpy as np
import ml_dtypes
from contextlib import ExitStack

import concourse.bass as bass
import concourse.bacc as bacc
import concourse.tile as tile
from concourse import mybir
from concourse.bass_utils import run_bass_kernel_spmd

F32 = mybir.dt.float32
BF16 = mybir.dt.bfloat16
AF = mybir.ActivationFunctionType
OP = mybir.AluOpType

B, C, D, H, W = 8, 512, 512, 128, 128
HW = H * W                      # 16384
P = 128                         # partitions
CCH = C // P                    # 4 channel chunks
NCORES = 8
G = 8                           # hw groups
GW = HW // G                    # 2048 group width
SCALE = float(C) ** -0.5

PGW = 1024                      # psum logits group width (2 banks each)
NH = GW // PGW                  # 2 psum halves per group

ROUTES = ("stt", "tt_act", "tt_pe", "tt_pe")
WQKB_W = CCH * C + 2 * CCH + P  # wqk chunks | ctx | bqk | identity


def _build_kernel():
    nc = bacc.Bacc(
        "TRN2",
        target_bir_lowering=False,
        debug=False,
        enable_asserts=False,
        num_devices=NCORES,
    )

    xd = nc.dram_tensor("xb", [C, HW], BF16, kind="ExternalInput")
    wqkbd = nc.dram_tensor("wqkb", [P, WQKB_W], BF16, kind="ExternalInput")
    wvtd = nc.dram_tensor("wvt", [P, CCH * C], BF16, kind="ExternalInput")
    bvd = nc.dram_tensor("bvc", [P, CCH], F32, kind="ExternalInput")
    outd = nc.dram_tensor("out", [C, HW], BF16, kind="ExternalOutput")

    pe_ccs = [cc for cc, r in enumerate(ROUTES) if r == "tt_pe"]

    with tile.TileContext(nc) as tc, ExitStack() as ctx:
        singles = ctx.enter_context(tc.tile_pool(name="singles", bufs=1))
        xt = ctx.enter_context(tc.tile_pool(name="xt", bufs=G * CCH))
        prods = ctx.enter_context(tc.tile_pool(name="prods", bufs=2))
        outp = ctx.enter_context(tc.tile_pool(name="outp", bufs=5))
        psb = ctx.enter_context(tc.tile_pool(name="psb", bufs=3))
        finp = ctx.enter_context(tc.tile_pool(name="finp", bufs=1))

        # ---- weights prefix on the scalar queue; x stream owns sync ----
        wqkb_sb = singles.tile([P, WQKB_W], BF16, tag="wqkb", name="wqkb")
        nc.scalar.dma_start(wqkb_sb[:], wqkbd[:])
        wvt_sb = singles.tile([P, CCH * C], BF16, tag="wvt", name="wvt")
        nc.scalar.dma_start(wvt_sb[:], wvtd[:])
        bv_sb = singles.tile([P, CCH], F32, tag="bv", name="bv")
        nc.scalar.dma_start(bv_sb[:], bvd[:])
        ident = wqkb_sb[:, CCH * C + 2 * CCH:CCH * C + 2 * CCH + P]

        ones128 = singles.tile([P, P], BF16, tag="ones128")
        nc.vector.memset(ones128[:], 1.0)

        qks_sb = singles.tile([P, CCH], F32, tag="qks")
        qksb = [singles.tile([P, P], BF16, tag=f"qksb{cc}", name=f"qksb{cc}") for cc in range(CCH)]
        pooled_sb = singles.tile([P, CCH], BF16, tag="pooled")
        pooled_f32 = singles.tile([P, CCH], F32, tag="pooledf")
        gate_sb = singles.tile([P, CCH], F32, tag="gate")
        zcols = singles.tile([P, G * NH], F32, tag="zcols")
        pcols = [singles.tile([P, G], F32, tag=f"pcols{cc}", name=f"pcols{cc}") for cc in range(CCH)]
        z_sb = singles.tile([P, 1], F32, tag="z")
        z2_sb = singles.tile([P, 1], F32, tag="z2")
        pp_sb = singles.tile([P, CCH], F32, tag="ppart")
        rz_sb = singles.tile([P, 1], F32, tag="rz")

        # ---- x stream: bulk on the sync queue; the tail groups split into
        # half-tiles across both queues so the ring-drain runt is small ----
        # one sync ring, issued in processing order: the ring's slow
        # tail-drain (runt) then only delays the group processed last.
        GORDER = [0, 1, 2, 3, 4, 7, 6, 5]
        x_tiles = {}
        for g in GORDER:
            for cc in range(CCH):
                t = xt.tile([P, GW], BF16, tag="x", name="x_t")
                nc.sync.dma_start(t[:], xd[cc * P:(cc + 1) * P, g * GW:(g + 1) * GW])
                x_tiles[(cc, g)] = t

        # ---- qks = ctx @ Wqk + bqk (pre-scaled); qksB replicated 128x ----
        with tc.tile_pool(name="pssm1", bufs=2, space="PSUM") as pssm1:
            for cc in range(CCH):
                pqk = pssm1.tile([P, 1], F32, tag="pssm", name="pssm_t")
                for dc in range(CCH):
                    nc.tensor.matmul(
                        pqk[:], wqkb_sb[:, dc * C + cc * P:dc * C + (cc + 1) * P],
                        wqkb_sb[:, CCH * C + dc:CCH * C + dc + 1],
                        start=(dc == 0), stop=(dc == CCH - 1),
                    )
                nc.vector.tensor_add(
                    qks_sb[:, cc:cc + 1], pqk[:],
                    wqkb_sb[:, CCH * C + CCH + cc:CCH * C + CCH + cc + 1])
                nc.vector.tensor_scalar_mul(qksb[cc][:], ones128[:], qks_sb[:, cc:cc + 1])

        # ---- fused pass: logits -> exp -> products, one group pipelined ----
        with tc.tile_pool(name="pslog", bufs=2, space="PSUM") as pslog, \
             tc.tile_pool(name="psacc", bufs=1, space="PSUM") as psacc:
            accs = {cc: psacc.tile([P, 2 * 512], F32, tag=f"acc{cc}", name=f"acc{cc}")
                    for cc in pe_ccs}
            half_mm_count = {(cc, hb): 0 for cc in pe_ccs for hb in (0, 1)}
            HALF_MM_TOTAL = G * (GW // 512) // 2

            pr_tiles = {}
            pe_tiles = {}

            def _act_reduce(pos):
                for cc in range(CCH):
                    if (cc, pos) in pr_tiles:
                        pr = pr_tiles.pop((cc, pos))
                        nc.scalar.activation(
                            pr[:], pr[:], AF.Copy,
                            accum_out=pcols[cc][:, pos:pos + 1],
                        )

            def _pe_acc(pos):
                for s in range(GW // 512):
                    hb = s % 2
                    for cc in pe_ccs:
                        pr = pe_tiles[(cc, pos)]
                        i = half_mm_count[(cc, hb)]
                        nc.tensor.matmul(
                            accs[cc][:, hb * 512:(hb + 1) * 512],
                            ident, pr[:, s * 512:(s + 1) * 512],
                            start=(i == 0), stop=(i == HALF_MM_TOTAL - 1),
                            skip_group_check=True,
                        )
                        half_mm_count[(cc, hb)] += 1
                for cc in pe_ccs:
                    pe_tiles.pop((cc, pos))

            prev_g = None
            for pos, g in enumerate(GORDER):
                p_t = psb.tile([P, GW], BF16, tag="p", name="p_t")
                for h in range(NH):
                    gh = pos * NH + h
                    plog = pslog.tile([P, PGW], F32, tag="plog", name="plog_t")
                    for s in range(PGW // 512):
                        for cc in range(CCH):
                            nc.tensor.matmul(
                                plog[:, s * 512:(s + 1) * 512],
                                qksb[cc][:],
                                x_tiles[(cc, g)][:, h * PGW + s * 512:h * PGW + (s + 1) * 512],
                                start=(cc == 0), stop=(cc == CCH - 1),
                            )
                    nc.scalar.activation(
                        p_t[:, h * PGW:(h + 1) * PGW], plog[:], AF.Exp,
                        accum_out=zcols[:, gh:gh + 1],
                    )
                routes = list(ROUTES)
                if pos in (1, 3):
                    routes[0] = "tt_act"      # rebalance DVE->ACT early on
                if pos == G - 1:
                    routes[1] = "stt"         # no ACT copy in the drain
                for cc in range(CCH):
                    if routes[cc] != "stt":
                        pr = prods.tile([P, GW], BF16, tag=f"pr{cc}", name=f"pr{cc}_t")
                        nc.vector.tensor_mul(pr[:], x_tiles[(cc, g)][:], p_t[:])
                        if routes[cc] == "tt_act":
                            pr_tiles[(cc, pos)] = pr
                        else:
                            pe_tiles[(cc, pos)] = pr
                stt_ccs = [cc for cc in range(CCH) if routes[cc] == "stt"]
                for k, cc in enumerate(stt_ccs):
                    # the LAST stt's (discarded) output overwrites the dead
                    # p_t; earlier ones write a scratch prods tile
                    if k == len(stt_ccs) - 1:
                        gout = p_t
                    else:
                        gout = prods.tile([P, GW], BF16, tag="pr1", name="pr1_t")
                    nc.vector.scalar_tensor_tensor(
                        gout[:], x_tiles[(cc, g)][:], 1.0, p_t[:],
                        op0=OP.mult, op1=OP.mult,
                        accum_out=pcols[cc][:, pos:pos + 1],
                    )
                if prev_g is not None:
                    _act_reduce(prev_g)
                    _pe_acc(prev_g)
                if pos == G - 1:
                    # all positions but the last have their pcols columns
                    # written now; pre-reduce them during the last chain
                    for cc in range(CCH):
                        if ROUTES[cc] in ("stt", "tt_act"):
                            nc.vector.reduce_sum(
                                pp_sb[:, cc:cc + 1], pcols[cc][:, 0:G - 1],
                                axis=mybir.AxisListType.X)
                if pos == G - 2:
                    # pre-reduce the first seven positions while the last
                    # group is still streaming/processing
                    nc.vector.reduce_sum(
                        z_sb[:], zcols[:, 0:(G - 1) * NH], axis=mybir.AxisListType.X)
                    for cc in range(CCH):
                        if ROUTES[cc] in ("stt", "tt_act"):
                            nc.vector.reduce_sum(
                                pp_sb[:, cc:cc + 1], pcols[cc][:, 0:G - 1],
                                axis=mybir.AxisListType.X)
                prev_g = pos
            _act_reduce(prev_g)
            _pe_acc(prev_g)

            # ---- finalize: Z, pooled ----
            nc.vector.reduce_sum(
                z2_sb[:], zcols[:, (G - 1) * NH:], axis=mybir.AxisListType.X)
            nc.vector.tensor_add(z_sb[:], z_sb[:], z2_sb[:])
            nc.vector.reciprocal(rz_sb[:], z_sb[:])
            for cc in range(CCH):
                if ROUTES[cc] == "tt_pe":
                    sa = finp.tile([P, 2 * 512], F32, tag="fin", name="fin_t")
                    nc.scalar.activation(
                        sa[:], accs[cc][:], AF.Copy,
                        accum_out=pooled_f32[:, cc:cc + 1],
                    )
                else:
                    nc.vector.tensor_add(
                        pooled_f32[:, cc:cc + 1], pp_sb[:, cc:cc + 1],
                        pcols[cc][:, G - 1:G],
                    )
            nc.vector.tensor_copy(pooled_sb[:], pooled_f32[:])

        # ---- gate = (Wv @ pooled)/Z + bv ----
        with tc.tile_pool(name="pssm2", bufs=1, space="PSUM") as pssm2:
            pg = pssm2.tile([P, CCH], F32, tag="pssm", name="pssm_t")
            for oc in range(CCH):
                for cc in range(CCH):
                    nc.tensor.matmul(
                        pg[:, oc:oc + 1],
                        wvt_sb[:, cc * C + oc * P:cc * C + (oc + 1) * P],
                        pooled_sb[:, cc:cc + 1],
                        start=(cc == 0), stop=(cc == CCH - 1),
                        skip_group_check=True,
                    )
            nc.vector.scalar_tensor_tensor(
                gate_sb[:], pg[:], rz_sb[:], bv_sb[:],
                op0=OP.mult, op1=OP.add,
            )

        # ---- pass C: out = x * gate (all of x still resident in SBUF) ----
        engs = [nc.sync, nc.scalar]
        for idx in range(G * CCH):
            g, cc = divmod(idx, CCH)
            o = outp.tile([P, GW], BF16, tag="o", name="o_t")
            nc.vector.tensor_scalar_mul(o[:], x_tiles[(cc, g)][:], gate_sb[:, cc:cc + 1])
            if idx < G * CCH - 2:
                engs[idx % 2].dma_start(outd[cc * P:(cc + 1) * P, g * GW:(g + 1) * GW], o[:])
            else:
                # halve the final transfers so the ring-drain runt is short
                for hh in range(2):
                    engs[(idx + hh) % 2].dma_start(
                        outd[cc * P:(cc + 1) * P,
                             g * GW + hh * (GW // 2):g * GW + (hh + 1) * (GW // 2)],
                        o[:, hh * (GW // 2):(hh + 1) * (GW // 2)])

    nc.compile()
    return nc


_NC = None


def _get_nc():
    global _NC
    if _NC is None:
        _NC = _build_kernel()
    return _NC


def _chunk_major(w):
    # w_c[p, j*C + k] = w[j*128 + p, k]
    w = np.asarray(w, dtype=np.float32).reshape(CCH, P, C)
    return np.ascontiguousarray(w.transpose(1, 0, 2).reshape(P, CCH * C))


def _make_in_maps(x, context, Wq, bq, Wk, bk, Wv, bv):
    bf = ml_dtypes.bfloat16
    x = np.asarray(x, dtype=np.float32).reshape(B, C, HW).astype(bf)
    Wq = np.asarray(Wq, dtype=np.float32)
    Wk = np.asarray(Wk, dtype=np.float32)
    wqk = _chunk_major(Wq.T @ Wk * SCALE).astype(bf)
    bqk = (np.asarray(bq, dtype=np.float32) @ Wk) * SCALE
    wvt = _chunk_major(np.asarray(Wv, dtype=np.float32).T).astype(bf)
    bqkc = np.ascontiguousarray(bqk.reshape(CCH, P).T).astype(bf)
    bvc = np.ascontiguousarray(np.asarray(bv, dtype=np.float32).reshape(CCH, P).T)
    identc = np.eye(P, dtype=np.float32).astype(bf)
    context = np.asarray(context, dtype=np.float32)
    in_maps = []
    for b in range(NCORES):
        ctxc = np.ascontiguousarray(context[b].reshape(CCH, P).T).astype(bf)
        wqkb = np.ascontiguousarray(
            np.concatenate([wqk, ctxc, bqkc, identc], axis=1))
        in_maps.append({
            "xb": x[b],
            "wqkb": wqkb,
            "wvt": wvt,
            "bvc": bvc,
        })
    return in_maps


def run_spmd(x, context, Wq, bq, Wk, bk, Wv, bv, **spmd_kwargs):
    """Run on 8 NeuronCores; returns (output (B,C,H,W) f32, BassKernelResults)."""
    nc = _get_nc()
    in_maps = _make_in_maps(x, context, Wq, bq, Wk, bk, Wv, bv)
    res = run_bass_kernel_spmd(nc, in_maps, list(range(NCORES)), **spmd_kwargs)
    out = np.stack([
        np.asarray(res.results[b]["out"]).astype(np.float32).reshape(C, H, W)
        for b in range(NCORES)
    ])
    return out, res


def kernel(x, context, Wq, bq, Wk, bk, Wv, bv):
    out, _ = run_spmd(x, context, Wq, bq, Wk, bk, Wv, bv)
    return out
